# revision 11
# baseline (speedup 1.0000x reference)
"""Trainium2 Bass kernel for the AF3-style diffusion loss.

Contract: kernel(**inputs) takes the FULL inputs (as in reference.setup_inputs)
and returns the FULL scalar output.

Strategy (8 NeuronCores):
  - Data-parallel over batch (B=2) x 4 row-blocks of 512 atoms -> 8 shards.
  - Each core computes, for its 512x2048 slice of the pairwise matrices:
      s15[l]  = sum_j (d_gt < 15)
      s30[l]  = sum_j (d_gt < 30)
      s15e[l] = sum_j (d_gt < 15) * e4      (e4 = sum of 4 sigmoids, unscaled)
      s30e[l] = sum_j (d_gt < 30) * e4
      blk[l,k] = sum_{j in 8-block k} (dx-dgt)^2
  - d^2 = |xi|^2+|xj|^2-2 xi.xj via K=13 fp16 split-precision PE matmuls
    into PSUM (x = hi + lo per coordinate; hi*hi + lo*hi + hi*lo terms, and
    hi/lo-split norms; ~1e-3 absolute error on d^2). fp32 matmul is NOT
    used: the self-loading fp32 weight path crashes this runtime
    (NRT_EXEC_UNIT_UNRECOVERABLE) / returns zeros. sqrt on ACT (PSUM->SBUF).
    Row norms carry +1e-3 so d^2 > 0 (vs reference eps 1e-12 inside sqrt;
    both dx and dgt shift identically so sd/thresholds are unaffected).
  - ACT work is phased per rep (all Sqrt, then Abs/Sigmoid/Square) because
    no activation table holds both Sqrt and Sigmoid - one table reload each
    way per rep instead of one per tile.
  - The sigmoid/threshold pipeline runs in bf16 (values in [0,1] / {0,1});
    all row-sum accumulators (tensor_scalar accum_out, tensor_tensor_reduce
    accum_out) and the bond terms stay fp32.
  - Host (numpy, O(N) / O(T^2) only): token one-hot features, bond weights,
    denominators, diagonal corrections, the 3x3 Kabsch solve + weighted MSE,
    and the final combine.
"""

import os
import numpy as np

B, A, T, APT = 2, 2048, 256, 8
NCORES = 8
RB = A // 4          # 512 rows per core
OUTW = 16 + 1024     # 4x (s15,s30,s15e,s30e) + 4x 256 bond-block sums
SIGMA_DATA = 16.0
E0 = 0.25 * sum(1.0 / (1.0 + np.exp(-z)) for z in (0.5, 1.0, 2.0, 4.0))

_CACHE = {}
LAST_RESULTS = None  # test.py reads exec_time_ns from here


def _build_bass(reps=1):
    """PE-matmul distances + phased ACT + bf16 DVE downstream.
    ~100 instructions per rep; DVE/ACT each ~50us busy per rep in the
    CoreSim cost model, PE ~25us, pipelined across tiles and reps."""
    import concourse.bacc as bacc
    import concourse.mybir as mybir
    from concourse.tile import TileContext

    f32 = mybir.dt.float32
    bf16 = mybir.dt.bfloat16
    fp16 = mybir.dt.float16
    Alu = mybir.AluOpType
    AF = mybir.ActivationFunctionType
    X = mybir.AxisListType.X

    nc = bacc.Bacc(None, target_bir_lowering=False)
    MMW = 2 * RB + 2 * A
    mm_d = nc.dram_tensor("mm", [13, MMW], fp16, kind="ExternalInput")
    out_d = nc.dram_tensor("out", [128, OUTW], f32, kind="ExternalOutput")

    with TileContext(nc) as tc:
        with (
            tc.tile_pool(name="cpool", bufs=1) as cp,
            tc.tile_pool(name="qpool", bufs=2) as qp,
            tc.tile_pool(name="opool", bufs=2) as op_,
            tc.tile_pool(name="ppool", bufs=2, space="PSUM") as pp,
        ):
            def act_const(val, nm):
                st = cp.tile([128, 1], f32, name=nm + "_st", tag=nm + "_st")
                nc.vector.memset(st[:], val)
                fin = cp.tile([128, 1], f32, name=nm, tag=nm)
                nc.scalar.activation(fin[:], st[:], AF.Copy)
                return fin
            bias0 = act_const(0.0, "bias0")
            btau = [act_const(float(tau), f"btau{k}")
                    for k, tau in enumerate((0.5, 1.0, 2.0, 4.0))]

            # persistent compute tiles; per-slice hazards are tracked
            # address-precisely by the Tile framework
            mm_sb = cp.tile([13, MMW], fp16, name="mm_sb", tag="mm_sb")
            da = cp.tile([128, 2, 4, 2048], f32, name="da", tag="da")
            sd = cp.tile([128, 4, 2048], f32, name="sd", tag="sd")

            for rep in range(reps):
                nc.sync.dma_start(mm_sb[:], mm_d[:])
                outb = op_.tile([128, OUTW], f32, name="out_sb", tag="out_sb")

                # ---- phase A: d = sqrt(matmul) for all 8 (si, t) tiles ----
                # tile_wait_until floors keep the Tile scheduler from
                # interleaving phase-B Sigmoid work into the Sqrt run (no
                # activation table holds both -> each interleave would cost
                # a 1.28us table reload on ACT).
                with tc.tile_wait_until(rep * 0.2):
                    for t in range(4):
                        for si in range(2):
                            lhsT = mm_sb[:, si * RB + t * 128: si * RB + (t + 1) * 128]
                            ps = pp.tile([128, 2048], f32, name="ps", tag="ps")
                            for ch in range(4):
                                rhs = mm_sb[:, 2 * RB + si * A + ch * 512:
                                            2 * RB + si * A + (ch + 1) * 512]
                                nc.tensor.matmul(ps[:, ch * 512:(ch + 1) * 512],
                                                 lhsT, rhs, start=True, stop=True)
                            nc.scalar.activation(da[:, si, t, :], ps[:], AF.Sqrt,
                                                 bias=bias0[:])

                # ---- phase B: per-tile lddt/bond pipeline ----
                # (tensor_tensor_reduce is avoided: it crashes this runtime.
                # scalar_tensor_tensor fuses mask+multiply+row-sum instead.)
                for t in range(4):
                    dx_t = da[:, 0, t, :]
                    dgt_t = da[:, 1, t, :]
                    sd_t = sd[:, t, :]
                    nc.vector.tensor_sub(sd_t, dgt_t, dx_t)
                    scr = qp.tile([128, 2048], bf16, name="scr", tag="scr")
                    nc.vector.tensor_scalar(scr[:], dgt_t, 15.0, None, Alu.is_lt,
                                            Alu.add, accum_out=outb[:, t:t + 1])
                    nc.vector.tensor_scalar(scr[:], dgt_t, 30.0, None, Alu.is_lt,
                                            Alu.add, accum_out=outb[:, 4 + t:5 + t])
                    ab = qp.tile([128, 2048], bf16, name="ab", tag="ab")
                    sg = qp.tile([128, 4, 2048], bf16, name="sg", tag="sg")
                    with tc.tile_wait_until(rep * 0.2 + 0.1):
                        nc.scalar.activation(ab[:], sd_t, AF.Abs, bias=bias0[:])
                        for k in range(4):
                            nc.scalar.activation(sg[:, k, :], ab[:], AF.Sigmoid,
                                                 bias=btau[k][:], scale=-1.0)
                    nc.vector.tensor_add(sg[:, 0, :], sg[:, 0, :], sg[:, 1, :])
                    nc.vector.tensor_add(sg[:, 2, :], sg[:, 2, :], sg[:, 3, :])
                    nc.vector.tensor_add(sg[:, 0, :], sg[:, 0, :], sg[:, 2, :])
                    nc.vector.scalar_tensor_tensor(
                        scr[:], dgt_t, 15.0, sg[:, 0, :], Alu.is_lt, Alu.mult,
                        accum_out=outb[:, 8 + t:9 + t])
                    nc.vector.scalar_tensor_tensor(
                        scr[:], dgt_t, 30.0, sg[:, 0, :], Alu.is_lt, Alu.mult,
                        accum_out=outb[:, 12 + t:13 + t])
                    d2 = qp.tile([128, 2048], f32, name="d2", tag="d2")
                    with tc.tile_wait_until(rep * 0.2 + 0.1):
                        nc.scalar.activation(d2[:], sd_t, AF.Square, bias=bias0[:])
                    d2v = d2[:].rearrange("p (k e) -> p k e", e=APT)
                    nc.vector.tensor_reduce(
                        outb[:, 16 + 256 * t:16 + 256 * (t + 1)], d2v,
                        axis=X, op=Alu.add)

                nc.sync.dma_start(out_d[:], outb[:])
    nc.compile()
    return nc


def _tok_features(isp, isd, isr, isl, tb, tm, npt):
    """Token->atom features, general in npt/tm. All numpy, O(A*T)."""
    cum = np.cumsum(npt, -1)
    start = cum - npt
    l = np.arange(A)
    ind = ((l[:, None] >= start[:, None, :]) & (l[:, None] < cum[:, None, :]))
    ind = ind.astype(np.float32)                      # [B,A,T] pure indicator
    oh = ind * tm[:, None, :]
    is_nuc = np.einsum('blt,bt->bl', oh, isd + isr)
    w_tok = 1.0 + isd * 5.0 + isr * 5.0 + isl * 10.0
    w_atom = np.einsum('blt,bt->bl', oh, w_tok)
    is_poly = isp + isd + isr
    tbm = tb * (is_poly[:, None, :] * isl[:, :, None]) * tm[:, None, :] * tm[:, :, None]
    wb_full = np.einsum('blt,btj->blj', ind, tbm)     # [B,A,T] bond row weights
    return oh, ind, is_nuc, w_atom, tbm, wb_full


def _mse_host(x, gt, gm, w_atom):
    """Weighted rigid align (Kabsch) of gt onto x + weighted MSE. Per sample."""
    denom = gm.sum()
    w_mean = (w_atom * gm).sum() / denom
    wm = (w_atom * gm)[:, None]
    mu = (gt * wm).sum(0) / denom / w_mean
    mu_gt = (x * wm).sum(0) / denom / w_mean
    xc = gt - mu
    xgc = x - mu_gt
    H = (xgc * wm).T @ xc
    U, _, Vh = np.linalg.svd(H)
    det = np.linalg.det(U @ Vh)
    s = np.array([1.0, 1.0, np.sign(det)])
    R = U @ (Vh * s[:, None])
    gt_al = xc @ R.T + mu_gt
    return (1.0 / 3.0) * (((x - gt_al) ** 2).sum(-1) * w_atom * gm).sum() / denom


def _numpy_fallback(x, gt, gm, isp, isd, isr, isl, tb, tm, npt, t):
    """Full-precision numpy port of the reference; used only when the inputs
    fall outside the fast-path assumptions (non-uniform atoms/masks)."""
    oh, ind, is_nuc, w_atom, tbm, wb_full = _tok_features(isp, isd, isr, isl, tb, tm, npt)
    sig = lambda z: 1.0 / (1.0 + np.exp(-z))
    loss = 0.0
    for b in range(B):
        d = x[b][:, None, :] - x[b][None, :, :]
        dx = np.sqrt((d * d).sum(-1) + 1e-12)
        d = gt[b][:, None, :] - gt[b][None, :, :]
        dg = np.sqrt((d * d).sum(-1) + 1e-12)
        pm = gm[b][:, None] * gm[b][None, :]
        bm = ind[b] @ tbm[b] @ ind[b].T
        m = bm * pm
        lb = (((dx - dg) ** 2) * m).sum() / m.sum()
        dd = np.abs(dg - dx)
        e = 0.25 * (sig(0.5 - dd) + sig(1.0 - dd) + sig(2.0 - dd) + sig(4.0 - dd))
        c = (dg < 30) * is_nuc[b][:, None] + (dg < 15) * (1.0 - is_nuc[b][:, None])
        m2 = (1.0 - np.eye(A)) * pm
        msum = m2.sum()
        ll = 1.0 - ((c * e * m2).sum() / msum) / ((c * m2).sum() / msum)
        lm = _mse_host(x[b], gt[b], gm[b], w_atom[b])
        wt = (t[b] ** 2 + SIGMA_DATA ** 2) / (t[b] + SIGMA_DATA) ** 2
        loss += wt * (lm + lb) + ll
    return np.float32(loss / B)


def kernel(x, gt_atom_positions, gt_atom_mask, is_protein, is_dna, is_rna,
           is_ligand, token_bonds, token_mask, num_atoms_per_token, t):
    global LAST_RESULTS
    f = np.asarray
    x = f(x, np.float32)
    gt = f(gt_atom_positions, np.float32)
    gm = f(gt_atom_mask, np.float32)
    isp, isd, isr, isl = (f(v, np.float32) for v in
                          (is_protein, is_dna, is_rna, is_ligand))
    tb = f(token_bonds, np.float32)
    tm = f(token_mask, np.float32)
    npt = f(num_atoms_per_token, np.int32)
    t = f(t, np.float32)

    fast = bool(np.all(npt == APT)) and bool(np.all(gm == 1.0))
    if not fast:
        return _numpy_fallback(x, gt, gm, isp, isd, isr, isl, tb, tm, npt, t)

    oh, ind, is_nuc, w_atom, tbm, wb_full = _tok_features(isp, isd, isr, isl, tb, tm, npt)

    # Per-core device inputs: core c -> sample b=c//4, rows [512r, 512r+512)
    # fp16 split packing: d^2 = sum_k lhsT[k]*rhs[k] over K=13 rows
    #   k 0-2 : (-2 x_r)_hi * (x_c)_hi      k 9 : nr_hi * 1
    #   k 3-5 : (-2 x_r)_lo * (x_c)_hi      k 10: nr_lo * 1
    #   k 6-8 : (-2 x_r)_hi * (x_c)_lo      k 11: 1 * nc_hi
    #                                       k 12: 1 * nc_lo
    f16 = np.float16

    def split(v):
        hi = v.astype(f16)
        lo = (v - hi.astype(np.float32)).astype(f16)
        return hi, lo

    in_maps = []
    for c in range(NCORES):
        b, r = divmod(c, 4)
        rows = slice(RB * r, RB * (r + 1))
        xb, gb = x[b], gt[b]
        ni = (xb * xb).sum(-1)
        gi = (gb * gb).sum(-1)

        def packs(coords, nrm, sl):
            m = np.empty((13, RB), f16)
            rh, rl = split(-2.0 * coords[sl].T)
            nh, nl = split(nrm[sl] + 1e-3)  # keeps d^2 > 0 under cancellation
            m[0:3] = rh; m[3:6] = rl; m[6:9] = rh
            m[9] = nh; m[10] = nl; m[11] = 1.0; m[12] = 1.0
            return m

        def packr(coords, nrm):
            m = np.empty((13, A), f16)
            ch, cl = split(coords.T)
            nh, nl = split(nrm)
            m[0:3] = ch; m[3:6] = ch; m[6:9] = cl
            m[9] = 1.0; m[10] = 1.0; m[11] = nh; m[12] = nl
            return m

        mm = np.empty((13, 2 * RB + 2 * A), f16)
        mm[:, 0:RB] = packs(xb, ni, rows)
        mm[:, RB:2 * RB] = packs(gb, gi, rows)
        mm[:, 2 * RB:2 * RB + A] = packr(xb, ni)
        mm[:, 2 * RB + A:2 * RB + 2 * A] = packr(gb, gi)
        in_maps.append({"mm": mm})

    if "nc" not in _CACHE:
        _CACHE["nc"] = _build_bass()
    os.environ.setdefault("BASS_NEVER_TRACE", "1")
    from concourse.bass_utils import run_bass_kernel_spmd
    res = run_bass_kernel_spmd(_CACHE["nc"], in_maps, core_ids=list(range(NCORES)))
    LAST_RESULTS = res
    globals()["LAST_IN_MAPS"] = in_maps

    # Host combine. Device layout: cols [0:4)=s15, [4:8)=s30, [8:12)=s15e,
    # [12:16)=s30e (col index = row-tile t), [16:16+1024) = bond 8-block
    # sums (256 per tile t). Row l = 512*r + 128*t + p.
    loss = 0.0
    for b in range(B):
        s15 = np.empty(A, np.float64); s30 = np.empty(A, np.float64)
        s15e = np.empty(A, np.float64); s30e = np.empty(A, np.float64)
        blk = np.empty((A, T), np.float64)
        for r in range(4):
            o = res.results[4 * b + r]["out"]  # [128, OUTW]
            for seg in range(4):
                base = RB * r + 128 * seg
                s15[base:base + 128] = o[:, seg]
                s30[base:base + 128] = o[:, 4 + seg]
                s15e[base:base + 128] = o[:, 8 + seg]
                s30e[base:base + 128] = o[:, 12 + seg]
                blk[base:base + 128] = o[:, 16 + seg * 256:16 + (seg + 1) * 256]
        bond = (blk * wb_full[b]).sum(-1)
        nuc = is_nuc[b].astype(np.float64)
        c_rows = s15 + nuc * (s30 - s15) - 1.0
        ce_rows = 0.25 * (s15e + nuc * (s30e - s15e)) - E0
        ll = 1.0 - ce_rows.sum() / c_rows.sum()
        a_i = ind[b].T @ gm[b].astype(np.float32)     # atoms per token (masked)
        bond_den = float(a_i @ tbm[b] @ a_i)
        lb = bond.sum() / bond_den
        lm = _mse_host(x[b], gt[b], gm[b], w_atom[b])
        wt = (t[b] ** 2 + SIGMA_DATA ** 2) / (t[b] + SIGMA_DATA) ** 2
        loss += wt * (lm + lb) + ll
    return np.float32(loss / B)


# revision 15
# speedup vs baseline: 51.4214x; 51.4214x over previous
"""Trainium2 Bass kernel for the AF3-style diffusion loss.

Contract: kernel(**inputs) takes the FULL inputs (as in reference.setup_inputs)
and returns the FULL scalar output.

Strategy (8 NeuronCores):
  - Data-parallel over batch (B=2) x 4 row-blocks of 512 atoms -> 8 shards.
  - Each core computes, for its 512x2048 slice of the pairwise matrices:
      s15[l]  = sum_j (d_gt < 15)
      s30[l]  = sum_j (d_gt < 30)
      s15e[l] = sum_j (d_gt < 15) * e4      (e4 = sum of 4 sigmoids, unscaled)
      s30e[l] = sum_j (d_gt < 30) * e4
      blk[l,k] = sum_{j in 8-block k} (dx-dgt)^2
  - d^2 = |xi|^2+|xj|^2-2 xi.xj via K=13 fp16 split-precision PE matmuls
    into PSUM (x = hi + lo per coordinate; hi*hi + lo*hi + hi*lo terms, and
    hi/lo-split norms; ~1e-3 absolute error on d^2). fp32 matmul is NOT
    used: the self-loading fp32 weight path crashes this runtime
    (NRT_EXEC_UNIT_UNRECOVERABLE) / returns zeros. sqrt on ACT (PSUM->SBUF).
    Row norms carry +1e-3 so d^2 > 0 (vs reference eps 1e-12 inside sqrt;
    both dx and dgt shift identically so sd/thresholds are unaffected).
  - ACT work is phased per rep (all Sqrt, then Abs/Sigmoid/Square) because
    no activation table holds both Sqrt and Sigmoid - one table reload each
    way per rep instead of one per tile.
  - The sigmoid/threshold pipeline runs in bf16 (values in [0,1] / {0,1});
    all row-sum accumulators (tensor_scalar accum_out, tensor_tensor_reduce
    accum_out) and the bond terms stay fp32.
  - Host (numpy, O(N) / O(T^2) only): token one-hot features, bond weights,
    denominators, diagonal corrections, the 3x3 Kabsch solve + weighted MSE,
    and the final combine.
"""

import os
import numpy as np

B, A, T, APT = 2, 2048, 256, 8
NCORES = 8
RB = A // 4          # 512 rows per core
OUTW = 16 + 1024     # 4x (s15,s30,s15e,s30e) + 4x 256 bond-block sums
SIGMA_DATA = 16.0
E0 = 0.25 * sum(1.0 / (1.0 + np.exp(-z)) for z in (0.5, 1.0, 2.0, 4.0))

_CACHE = {}
LAST_RESULTS = None  # test.py reads exec_time_ns from here

# experiment knobs (read once at build time)
FLOORS = os.environ.get("K_FLOORS", "1") == "1"
BOND_GPSIMD = False  # gpsimd tensor_reduce is partition-axis only
SD_POOL = os.environ.get("K_SD_POOL", "0") == "1"
ADD_POOL = os.environ.get("K_ADD_POOL", "0") == "1"
FLOOR_P = float(os.environ.get("K_FLOOR_P", "0.2"))
FLOOR_H = float(os.environ.get("K_FLOOR_H", "0.1"))



def _floor(tc, ms):
    import contextlib
    return tc.tile_wait_until(ms) if FLOORS else contextlib.nullcontext()

def _build_bass(reps=1):
    """PE-matmul distances + phased ACT + bf16 DVE downstream.
    ~100 instructions per rep; DVE/ACT each ~50us busy per rep in the
    CoreSim cost model, PE ~25us, pipelined across tiles and reps."""
    import concourse.bacc as bacc
    import concourse.mybir as mybir
    from concourse.tile import TileContext

    f32 = mybir.dt.float32
    bf16 = mybir.dt.bfloat16
    fp16 = mybir.dt.float16
    Alu = mybir.AluOpType
    AF = mybir.ActivationFunctionType
    X = mybir.AxisListType.X

    nc = bacc.Bacc(None, target_bir_lowering=False)
    MMW = 2 * RB + 2 * A
    mm_d = nc.dram_tensor("mm", [13, MMW], fp16, kind="ExternalInput")
    out_d = nc.dram_tensor("out", [128, OUTW], f32, kind="ExternalOutput")

    with TileContext(nc) as tc:
        with (
            tc.tile_pool(name="cpool", bufs=1) as cp,
            tc.tile_pool(name="qpool", bufs=2) as qp,
            tc.tile_pool(name="opool", bufs=2) as op_,
            tc.tile_pool(name="ppool", bufs=2, space="PSUM") as pp,
        ):
            def act_const(val, nm):
                st = cp.tile([128, 1], f32, name=nm + "_st", tag=nm + "_st")
                nc.vector.memset(st[:], val)
                fin = cp.tile([128, 1], f32, name=nm, tag=nm)
                nc.scalar.activation(fin[:], st[:], AF.Copy)
                return fin
            bias0 = act_const(0.0, "bias0")
            btau = [act_const(float(tau), f"btau{k}")
                    for k, tau in enumerate((0.5, 1.0, 2.0, 4.0))]

            # persistent compute tiles; per-slice hazards are tracked
            # address-precisely by the Tile framework
            mm_sb = cp.tile([13, MMW], fp16, name="mm_sb", tag="mm_sb")
            da = cp.tile([128, 2, 4, 2048], f32, name="da", tag="da")
            sd = cp.tile([128, 4, 2048], f32, name="sd", tag="sd")

            for rep in range(reps):
                nc.sync.dma_start(mm_sb[:], mm_d[:])
                outb = op_.tile([128, OUTW], f32, name="out_sb", tag="out_sb")

                # ---- phase A: d = sqrt(matmul) for all 8 (si, t) tiles ----
                # tile_wait_until floors keep the Tile scheduler from
                # interleaving phase-B Sigmoid work into the Sqrt run (no
                # activation table holds both -> each interleave would cost
                # a 1.28us table reload on ACT).
                with _floor(tc, rep * FLOOR_P):
                    for t in range(4):
                        for si in range(2):
                            lhsT = mm_sb[:, si * RB + t * 128: si * RB + (t + 1) * 128]
                            ps = pp.tile([128, 2048], f32, name="ps", tag="ps")
                            for ch in range(4):
                                rhs = mm_sb[:, 2 * RB + si * A + ch * 512:
                                            2 * RB + si * A + (ch + 1) * 512]
                                nc.tensor.matmul(ps[:, ch * 512:(ch + 1) * 512],
                                                 lhsT, rhs, start=True, stop=True)
                            nc.scalar.activation(da[:, si, t, :], ps[:], AF.Sqrt,
                                                 bias=bias0[:])

                # ---- phase B: per-tile lddt/bond pipeline ----
                # (tensor_tensor_reduce is avoided: it crashes this runtime.
                # scalar_tensor_tensor fuses mask+multiply+row-sum instead.)
                for t in range(4):
                    dx_t = da[:, 0, t, :]
                    dgt_t = da[:, 1, t, :]
                    sd_t = sd[:, t, :]
                    sd_eng = nc.gpsimd if SD_POOL else nc.vector
                    sd_eng.tensor_sub(sd_t, dgt_t, dx_t)
                    scr = qp.tile([128, 2048], bf16, name="scr", tag="scr")
                    nc.vector.tensor_scalar(scr[:], dgt_t, 15.0, None, Alu.is_lt,
                                            Alu.add, accum_out=outb[:, t:t + 1])
                    nc.vector.tensor_scalar(scr[:], dgt_t, 30.0, None, Alu.is_lt,
                                            Alu.add, accum_out=outb[:, 4 + t:5 + t])
                    ab = qp.tile([128, 2048], bf16, name="ab", tag="ab")
                    sg = qp.tile([128, 4, 2048], bf16, name="sg", tag="sg")
                    with _floor(tc, rep * FLOOR_P + FLOOR_H):
                        nc.scalar.activation(ab[:], sd_t, AF.Abs, bias=bias0[:])
                        for k in range(4):
                            nc.scalar.activation(sg[:, k, :], ab[:], AF.Sigmoid,
                                                 bias=btau[k][:], scale=-1.0)
                    add2_eng = nc.gpsimd if ADD_POOL else nc.vector
                    nc.vector.tensor_add(sg[:, 0, :], sg[:, 0, :], sg[:, 1, :])
                    add2_eng.tensor_add(sg[:, 2, :], sg[:, 2, :], sg[:, 3, :])
                    nc.vector.tensor_add(sg[:, 0, :], sg[:, 0, :], sg[:, 2, :])
                    nc.vector.scalar_tensor_tensor(
                        scr[:], dgt_t, 15.0, sg[:, 0, :], Alu.is_lt, Alu.mult,
                        accum_out=outb[:, 8 + t:9 + t])
                    nc.vector.scalar_tensor_tensor(
                        scr[:], dgt_t, 30.0, sg[:, 0, :], Alu.is_lt, Alu.mult,
                        accum_out=outb[:, 12 + t:13 + t])
                    d2 = qp.tile([128, 2048], f32, name="d2", tag="d2")
                    with _floor(tc, rep * FLOOR_P + FLOOR_H):
                        nc.scalar.activation(d2[:], sd_t, AF.Square, bias=bias0[:])
                    d2v = d2[:].rearrange("p (k e) -> p k e", e=APT)
                    red_eng = nc.gpsimd if BOND_GPSIMD else nc.vector
                    red_eng.tensor_reduce(
                        outb[:, 16 + 256 * t:16 + 256 * (t + 1)], d2v,
                        axis=X, op=Alu.add)

                nc.sync.dma_start(out_d[:], outb[:])
    nc.compile()
    return nc


def _tok_features(isp, isd, isr, isl, tb, tm, npt):
    """Token->atom features, general in npt/tm. All numpy, O(A*T)."""
    cum = np.cumsum(npt, -1)
    start = cum - npt
    l = np.arange(A)
    ind = ((l[:, None] >= start[:, None, :]) & (l[:, None] < cum[:, None, :]))
    ind = ind.astype(np.float32)                      # [B,A,T] pure indicator
    oh = ind * tm[:, None, :]
    is_nuc = np.einsum('blt,bt->bl', oh, isd + isr)
    w_tok = 1.0 + isd * 5.0 + isr * 5.0 + isl * 10.0
    w_atom = np.einsum('blt,bt->bl', oh, w_tok)
    is_poly = isp + isd + isr
    tbm = tb * (is_poly[:, None, :] * isl[:, :, None]) * tm[:, None, :] * tm[:, :, None]
    wb_full = np.einsum('blt,btj->blj', ind, tbm)     # [B,A,T] bond row weights
    return oh, ind, is_nuc, w_atom, tbm, wb_full


def _mse_host(x, gt, gm, w_atom):
    """Weighted rigid align (Kabsch) of gt onto x + weighted MSE. Per sample."""
    denom = gm.sum()
    w_mean = (w_atom * gm).sum() / denom
    wm = (w_atom * gm)[:, None]
    mu = (gt * wm).sum(0) / denom / w_mean
    mu_gt = (x * wm).sum(0) / denom / w_mean
    xc = gt - mu
    xgc = x - mu_gt
    H = (xgc * wm).T @ xc
    U, _, Vh = np.linalg.svd(H)
    det = np.linalg.det(U @ Vh)
    s = np.array([1.0, 1.0, np.sign(det)])
    R = U @ (Vh * s[:, None])
    gt_al = xc @ R.T + mu_gt
    return (1.0 / 3.0) * (((x - gt_al) ** 2).sum(-1) * w_atom * gm).sum() / denom


def _numpy_fallback(x, gt, gm, isp, isd, isr, isl, tb, tm, npt, t):
    """Full-precision numpy port of the reference; used only when the inputs
    fall outside the fast-path assumptions (non-uniform atoms/masks)."""
    oh, ind, is_nuc, w_atom, tbm, wb_full = _tok_features(isp, isd, isr, isl, tb, tm, npt)
    sig = lambda z: 1.0 / (1.0 + np.exp(-z))
    loss = 0.0
    for b in range(B):
        d = x[b][:, None, :] - x[b][None, :, :]
        dx = np.sqrt((d * d).sum(-1) + 1e-12)
        d = gt[b][:, None, :] - gt[b][None, :, :]
        dg = np.sqrt((d * d).sum(-1) + 1e-12)
        pm = gm[b][:, None] * gm[b][None, :]
        bm = ind[b] @ tbm[b] @ ind[b].T
        m = bm * pm
        lb = (((dx - dg) ** 2) * m).sum() / m.sum()
        dd = np.abs(dg - dx)
        e = 0.25 * (sig(0.5 - dd) + sig(1.0 - dd) + sig(2.0 - dd) + sig(4.0 - dd))
        c = (dg < 30) * is_nuc[b][:, None] + (dg < 15) * (1.0 - is_nuc[b][:, None])
        m2 = (1.0 - np.eye(A)) * pm
        msum = m2.sum()
        ll = 1.0 - ((c * e * m2).sum() / msum) / ((c * m2).sum() / msum)
        lm = _mse_host(x[b], gt[b], gm[b], w_atom[b])
        wt = (t[b] ** 2 + SIGMA_DATA ** 2) / (t[b] + SIGMA_DATA) ** 2
        loss += wt * (lm + lb) + ll
    return np.float32(loss / B)


def kernel(x, gt_atom_positions, gt_atom_mask, is_protein, is_dna, is_rna,
           is_ligand, token_bonds, token_mask, num_atoms_per_token, t):
    global LAST_RESULTS
    f = np.asarray
    x = f(x, np.float32)
    gt = f(gt_atom_positions, np.float32)
    gm = f(gt_atom_mask, np.float32)
    isp, isd, isr, isl = (f(v, np.float32) for v in
                          (is_protein, is_dna, is_rna, is_ligand))
    tb = f(token_bonds, np.float32)
    tm = f(token_mask, np.float32)
    npt = f(num_atoms_per_token, np.int32)
    t = f(t, np.float32)

    fast = bool(np.all(npt == APT)) and bool(np.all(gm == 1.0))
    if not fast:
        return _numpy_fallback(x, gt, gm, isp, isd, isr, isl, tb, tm, npt, t)

    oh, ind, is_nuc, w_atom, tbm, wb_full = _tok_features(isp, isd, isr, isl, tb, tm, npt)

    # Per-core device inputs: core c -> sample b=c//4, rows [512r, 512r+512)
    # fp16 split packing: d^2 = sum_k lhsT[k]*rhs[k] over K=13 rows
    #   k 0-2 : (-2 x_r)_hi * (x_c)_hi      k 9 : nr_hi * 1
    #   k 3-5 : (-2 x_r)_lo * (x_c)_hi      k 10: nr_lo * 1
    #   k 6-8 : (-2 x_r)_hi * (x_c)_lo      k 11: 1 * nc_hi
    #                                       k 12: 1 * nc_lo
    f16 = np.float16

    def split(v):
        hi = v.astype(f16)
        lo = (v - hi.astype(np.float32)).astype(f16)
        return hi, lo

    in_maps = []
    for c in range(NCORES):
        b, r = divmod(c, 4)
        rows = slice(RB * r, RB * (r + 1))
        xb, gb = x[b], gt[b]
        ni = (xb * xb).sum(-1)
        gi = (gb * gb).sum(-1)

        def packs(coords, nrm, sl):
            m = np.empty((13, RB), f16)
            rh, rl = split(-2.0 * coords[sl].T)
            nh, nl = split(nrm[sl] + 1e-3)  # keeps d^2 > 0 under cancellation
            m[0:3] = rh; m[3:6] = rl; m[6:9] = rh
            m[9] = nh; m[10] = nl; m[11] = 1.0; m[12] = 1.0
            return m

        def packr(coords, nrm):
            m = np.empty((13, A), f16)
            ch, cl = split(coords.T)
            nh, nl = split(nrm)
            m[0:3] = ch; m[3:6] = ch; m[6:9] = cl
            m[9] = 1.0; m[10] = 1.0; m[11] = nh; m[12] = nl
            return m

        mm = np.empty((13, 2 * RB + 2 * A), f16)
        mm[:, 0:RB] = packs(xb, ni, rows)
        mm[:, RB:2 * RB] = packs(gb, gi, rows)
        mm[:, 2 * RB:2 * RB + A] = packr(xb, ni)
        mm[:, 2 * RB + A:2 * RB + 2 * A] = packr(gb, gi)
        in_maps.append({"mm": mm})

    if "nc" not in _CACHE:
        _CACHE["nc"] = _build_bass()
    os.environ.setdefault("BASS_NEVER_TRACE", "1")
    from concourse.bass_utils import run_bass_kernel_spmd
    res = run_bass_kernel_spmd(_CACHE["nc"], in_maps, core_ids=list(range(NCORES)))
    LAST_RESULTS = res
    globals()["LAST_IN_MAPS"] = in_maps

    # Host combine. Device layout: cols [0:4)=s15, [4:8)=s30, [8:12)=s15e,
    # [12:16)=s30e (col index = row-tile t), [16:16+1024) = bond 8-block
    # sums (256 per tile t). Row l = 512*r + 128*t + p.
    loss = 0.0
    for b in range(B):
        s15 = np.empty(A, np.float64); s30 = np.empty(A, np.float64)
        s15e = np.empty(A, np.float64); s30e = np.empty(A, np.float64)
        blk = np.empty((A, T), np.float64)
        for r in range(4):
            o = res.results[4 * b + r]["out"]  # [128, OUTW]
            for seg in range(4):
                base = RB * r + 128 * seg
                s15[base:base + 128] = o[:, seg]
                s30[base:base + 128] = o[:, 4 + seg]
                s15e[base:base + 128] = o[:, 8 + seg]
                s30e[base:base + 128] = o[:, 12 + seg]
                blk[base:base + 128] = o[:, 16 + seg * 256:16 + (seg + 1) * 256]
        bond = (blk * wb_full[b]).sum(-1)
        nuc = is_nuc[b].astype(np.float64)
        c_rows = s15 + nuc * (s30 - s15) - 1.0
        ce_rows = 0.25 * (s15e + nuc * (s30e - s15e)) - E0
        ll = 1.0 - ce_rows.sum() / c_rows.sum()
        a_i = ind[b].T @ gm[b].astype(np.float32)     # atoms per token (masked)
        bond_den = float(a_i @ tbm[b] @ a_i)
        lb = bond.sum() / bond_den
        lm = _mse_host(x[b], gt[b], gm[b], w_atom[b])
        wt = (t[b] ** 2 + SIGMA_DATA ** 2) / (t[b] + SIGMA_DATA) ** 2
        loss += wt * (lm + lb) + ll
    return np.float32(loss / B)


# revision 16
# speedup vs baseline: 56.5201x; 1.0992x over previous
"""Trainium2 Bass kernel for the AF3-style diffusion loss.

Contract: kernel(**inputs) takes the FULL inputs (as in reference.setup_inputs)
and returns the FULL scalar output.

Strategy (8 NeuronCores):
  - Data-parallel over batch (B=2) x 4 row-blocks of 512 atoms -> 8 shards.
  - Each core computes, for its 512x2048 slice of the pairwise matrices:
      s15[l]  = sum_j (d_gt < 15)
      s30[l]  = sum_j (d_gt < 30)
      s15e[l] = sum_j (d_gt < 15) * e4      (e4 = sum of 4 sigmoids, unscaled)
      s30e[l] = sum_j (d_gt < 30) * e4
      blk[l,k] = sum_{j in 8-block k} (dx-dgt)^2
  - d^2 = |xi|^2+|xj|^2-2 xi.xj via K=13 fp16 split-precision PE matmuls
    into PSUM (x = hi + lo per coordinate; hi*hi + lo*hi + hi*lo terms, and
    hi/lo-split norms; ~1e-3 absolute error on d^2). fp32 matmul is NOT
    used: the self-loading fp32 weight path crashes this runtime
    (NRT_EXEC_UNIT_UNRECOVERABLE) / returns zeros. sqrt on ACT (PSUM->SBUF).
    Row norms carry +1e-3 so d^2 > 0 (vs reference eps 1e-12 inside sqrt;
    both dx and dgt shift identically so sd/thresholds are unaffected).
  - ACT work is phased per rep (all Sqrt, then Abs/Sigmoid/Square) because
    no activation table holds both Sqrt and Sigmoid - one table reload each
    way per rep instead of one per tile.
  - The sigmoid/threshold pipeline runs in bf16 (values in [0,1] / {0,1});
    all row-sum accumulators (tensor_scalar accum_out, tensor_tensor_reduce
    accum_out) and the bond terms stay fp32.
  - Host (numpy, O(N) / O(T^2) only): token one-hot features, bond weights,
    denominators, diagonal corrections, the 3x3 Kabsch solve + weighted MSE,
    and the final combine.
"""

import os
import numpy as np

B, A, T, APT = 2, 2048, 256, 8
NCORES = 8
RB = A // 4          # 512 rows per core
OUTW = 16 + 1024     # 4x (s15,s30,s15e,s30e) + 4x 256 bond-block sums
SIGMA_DATA = 16.0
E0 = 0.25 * sum(1.0 / (1.0 + np.exp(-z)) for z in (0.5, 1.0, 2.0, 4.0))

_CACHE = {}
LAST_RESULTS = None  # test.py reads exec_time_ns from here

# experiment knobs (read once at build time)
FLOORS = os.environ.get("K_FLOORS", "1") == "1"
BOND_GPSIMD = False  # gpsimd tensor_reduce is partition-axis only
SD_POOL = os.environ.get("K_SD_POOL", "0") == "1"
ADD_POOL = os.environ.get("K_ADD_POOL", "0") == "1"
FLOOR_P = float(os.environ.get("K_FLOOR_P", "0.2"))
FLOOR_H = float(os.environ.get("K_FLOOR_H", "0.1"))



def _floor(tc, ms):
    import contextlib
    return tc.tile_wait_until(ms) if FLOORS else contextlib.nullcontext()

def _build_bass(reps=1):
    """PE-matmul distances + phased ACT + bf16 DVE downstream.
    ~100 instructions per rep; DVE/ACT each ~50us busy per rep in the
    CoreSim cost model, PE ~25us, pipelined across tiles and reps."""
    import concourse.bacc as bacc
    import concourse.mybir as mybir
    from concourse.tile import TileContext

    f32 = mybir.dt.float32
    bf16 = mybir.dt.bfloat16
    fp16 = mybir.dt.float16
    Alu = mybir.AluOpType
    AF = mybir.ActivationFunctionType
    X = mybir.AxisListType.X

    nc = bacc.Bacc(None, target_bir_lowering=False)
    MMW = 2 * RB + 2 * A
    mm_d = nc.dram_tensor("mm", [13, MMW], fp16, kind="ExternalInput")
    out_d = nc.dram_tensor("out", [128, OUTW], f32, kind="ExternalOutput")

    with TileContext(nc) as tc:
        with (
            tc.tile_pool(name="cpool", bufs=1) as cp,
            tc.tile_pool(name="qpool", bufs=2) as qp,
            tc.tile_pool(name="opool", bufs=2) as op_,
            tc.tile_pool(name="ppool", bufs=2, space="PSUM") as pp,
        ):
            def act_const(val, nm):
                st = cp.tile([128, 1], f32, name=nm + "_st", tag=nm + "_st")
                nc.vector.memset(st[:], val)
                fin = cp.tile([128, 1], f32, name=nm, tag=nm)
                nc.scalar.activation(fin[:], st[:], AF.Copy)
                return fin
            bias0 = act_const(0.0, "bias0")
            btau = [act_const(float(tau), f"btau{k}")
                    for k, tau in enumerate((0.5, 1.0, 2.0, 4.0))]

            # persistent compute tiles; per-slice hazards are tracked
            # address-precisely by the Tile framework
            da = cp.tile([128, 2, 4, 2048], f32, name="da", tag="da")
            sd = cp.tile([128, 4, 2048], f32, name="sd", tag="sd")

            for rep in range(reps):
                # double-buffered so rep r+1's input DMA overlaps rep r's
                # compute instead of serializing behind its last matmul read
                mm_sb = op_.tile([13, MMW], fp16, name="mm_sb", tag="mm_sb")
                nc.sync.dma_start(mm_sb[:], mm_d[:])
                outb = op_.tile([128, OUTW], f32, name="out_sb", tag="out_sb")

                # ---- phase A: d = sqrt(matmul) for all 8 (si, t) tiles ----
                # tile_wait_until floors keep the Tile scheduler from
                # interleaving phase-B Sigmoid work into the Sqrt run (no
                # activation table holds both -> each interleave would cost
                # a 1.28us table reload on ACT).
                with _floor(tc, rep * FLOOR_P):
                    for t in range(4):
                        for si in range(2):
                            lhsT = mm_sb[:, si * RB + t * 128: si * RB + (t + 1) * 128]
                            ps = pp.tile([128, 2048], f32, name="ps", tag="ps")
                            for ch in range(4):
                                rhs = mm_sb[:, 2 * RB + si * A + ch * 512:
                                            2 * RB + si * A + (ch + 1) * 512]
                                nc.tensor.matmul(ps[:, ch * 512:(ch + 1) * 512],
                                                 lhsT, rhs, start=True, stop=True)
                            nc.scalar.activation(da[:, si, t, :], ps[:], AF.Sqrt,
                                                 bias=bias0[:])

                # ---- phase B: per-tile lddt/bond pipeline ----
                # (tensor_tensor_reduce is avoided: it crashes this runtime.
                # scalar_tensor_tensor fuses mask+multiply+row-sum instead.)
                for t in range(4):
                    dx_t = da[:, 0, t, :]
                    dgt_t = da[:, 1, t, :]
                    sd_t = sd[:, t, :]
                    sd_eng = nc.gpsimd if SD_POOL else nc.vector
                    sd_eng.tensor_sub(sd_t, dgt_t, dx_t)
                    scr = qp.tile([128, 2048], bf16, name="scr", tag="scr")
                    nc.vector.tensor_scalar(scr[:], dgt_t, 15.0, None, Alu.is_lt,
                                            Alu.add, accum_out=outb[:, t:t + 1])
                    nc.vector.tensor_scalar(scr[:], dgt_t, 30.0, None, Alu.is_lt,
                                            Alu.add, accum_out=outb[:, 4 + t:5 + t])
                    ab = qp.tile([128, 2048], bf16, name="ab", tag="ab")
                    sg = qp.tile([128, 4, 2048], bf16, name="sg", tag="sg")
                    with _floor(tc, rep * FLOOR_P + FLOOR_H):
                        nc.scalar.activation(ab[:], sd_t, AF.Abs, bias=bias0[:])
                        for k in range(4):
                            nc.scalar.activation(sg[:, k, :], ab[:], AF.Sigmoid,
                                                 bias=btau[k][:], scale=-1.0)
                    add2_eng = nc.gpsimd if ADD_POOL else nc.vector
                    nc.vector.tensor_add(sg[:, 0, :], sg[:, 0, :], sg[:, 1, :])
                    add2_eng.tensor_add(sg[:, 2, :], sg[:, 2, :], sg[:, 3, :])
                    nc.vector.tensor_add(sg[:, 0, :], sg[:, 0, :], sg[:, 2, :])
                    nc.vector.scalar_tensor_tensor(
                        scr[:], dgt_t, 15.0, sg[:, 0, :], Alu.is_lt, Alu.mult,
                        accum_out=outb[:, 8 + t:9 + t])
                    nc.vector.scalar_tensor_tensor(
                        scr[:], dgt_t, 30.0, sg[:, 0, :], Alu.is_lt, Alu.mult,
                        accum_out=outb[:, 12 + t:13 + t])
                    d2 = qp.tile([128, 2048], f32, name="d2", tag="d2")
                    with _floor(tc, rep * FLOOR_P + FLOOR_H):
                        nc.scalar.activation(d2[:], sd_t, AF.Square, bias=bias0[:])
                    d2v = d2[:].rearrange("p (k e) -> p k e", e=APT)
                    red_eng = nc.gpsimd if BOND_GPSIMD else nc.vector
                    red_eng.tensor_reduce(
                        outb[:, 16 + 256 * t:16 + 256 * (t + 1)], d2v,
                        axis=X, op=Alu.add)

                nc.sync.dma_start(out_d[:], outb[:])
    nc.compile()
    return nc


def _tok_features(isp, isd, isr, isl, tb, tm, npt):
    """Token->atom features, general in npt/tm. All numpy, O(A*T)."""
    cum = np.cumsum(npt, -1)
    start = cum - npt
    l = np.arange(A)
    ind = ((l[:, None] >= start[:, None, :]) & (l[:, None] < cum[:, None, :]))
    ind = ind.astype(np.float32)                      # [B,A,T] pure indicator
    oh = ind * tm[:, None, :]
    is_nuc = np.einsum('blt,bt->bl', oh, isd + isr)
    w_tok = 1.0 + isd * 5.0 + isr * 5.0 + isl * 10.0
    w_atom = np.einsum('blt,bt->bl', oh, w_tok)
    is_poly = isp + isd + isr
    tbm = tb * (is_poly[:, None, :] * isl[:, :, None]) * tm[:, None, :] * tm[:, :, None]
    wb_full = np.einsum('blt,btj->blj', ind, tbm)     # [B,A,T] bond row weights
    return oh, ind, is_nuc, w_atom, tbm, wb_full


def _mse_host(x, gt, gm, w_atom):
    """Weighted rigid align (Kabsch) of gt onto x + weighted MSE. Per sample."""
    denom = gm.sum()
    w_mean = (w_atom * gm).sum() / denom
    wm = (w_atom * gm)[:, None]
    mu = (gt * wm).sum(0) / denom / w_mean
    mu_gt = (x * wm).sum(0) / denom / w_mean
    xc = gt - mu
    xgc = x - mu_gt
    H = (xgc * wm).T @ xc
    U, _, Vh = np.linalg.svd(H)
    det = np.linalg.det(U @ Vh)
    s = np.array([1.0, 1.0, np.sign(det)])
    R = U @ (Vh * s[:, None])
    gt_al = xc @ R.T + mu_gt
    return (1.0 / 3.0) * (((x - gt_al) ** 2).sum(-1) * w_atom * gm).sum() / denom


def _numpy_fallback(x, gt, gm, isp, isd, isr, isl, tb, tm, npt, t):
    """Full-precision numpy port of the reference; used only when the inputs
    fall outside the fast-path assumptions (non-uniform atoms/masks)."""
    oh, ind, is_nuc, w_atom, tbm, wb_full = _tok_features(isp, isd, isr, isl, tb, tm, npt)
    sig = lambda z: 1.0 / (1.0 + np.exp(-z))
    loss = 0.0
    for b in range(B):
        d = x[b][:, None, :] - x[b][None, :, :]
        dx = np.sqrt((d * d).sum(-1) + 1e-12)
        d = gt[b][:, None, :] - gt[b][None, :, :]
        dg = np.sqrt((d * d).sum(-1) + 1e-12)
        pm = gm[b][:, None] * gm[b][None, :]
        bm = ind[b] @ tbm[b] @ ind[b].T
        m = bm * pm
        lb = (((dx - dg) ** 2) * m).sum() / m.sum()
        dd = np.abs(dg - dx)
        e = 0.25 * (sig(0.5 - dd) + sig(1.0 - dd) + sig(2.0 - dd) + sig(4.0 - dd))
        c = (dg < 30) * is_nuc[b][:, None] + (dg < 15) * (1.0 - is_nuc[b][:, None])
        m2 = (1.0 - np.eye(A)) * pm
        msum = m2.sum()
        ll = 1.0 - ((c * e * m2).sum() / msum) / ((c * m2).sum() / msum)
        lm = _mse_host(x[b], gt[b], gm[b], w_atom[b])
        wt = (t[b] ** 2 + SIGMA_DATA ** 2) / (t[b] + SIGMA_DATA) ** 2
        loss += wt * (lm + lb) + ll
    return np.float32(loss / B)


def kernel(x, gt_atom_positions, gt_atom_mask, is_protein, is_dna, is_rna,
           is_ligand, token_bonds, token_mask, num_atoms_per_token, t):
    global LAST_RESULTS
    f = np.asarray
    x = f(x, np.float32)
    gt = f(gt_atom_positions, np.float32)
    gm = f(gt_atom_mask, np.float32)
    isp, isd, isr, isl = (f(v, np.float32) for v in
                          (is_protein, is_dna, is_rna, is_ligand))
    tb = f(token_bonds, np.float32)
    tm = f(token_mask, np.float32)
    npt = f(num_atoms_per_token, np.int32)
    t = f(t, np.float32)

    fast = bool(np.all(npt == APT)) and bool(np.all(gm == 1.0))
    if not fast:
        return _numpy_fallback(x, gt, gm, isp, isd, isr, isl, tb, tm, npt, t)

    oh, ind, is_nuc, w_atom, tbm, wb_full = _tok_features(isp, isd, isr, isl, tb, tm, npt)

    # Per-core device inputs: core c -> sample b=c//4, rows [512r, 512r+512)
    # fp16 split packing: d^2 = sum_k lhsT[k]*rhs[k] over K=13 rows
    #   k 0-2 : (-2 x_r)_hi * (x_c)_hi      k 9 : nr_hi * 1
    #   k 3-5 : (-2 x_r)_lo * (x_c)_hi      k 10: nr_lo * 1
    #   k 6-8 : (-2 x_r)_hi * (x_c)_lo      k 11: 1 * nc_hi
    #                                       k 12: 1 * nc_lo
    f16 = np.float16

    def split(v):
        hi = v.astype(f16)
        lo = (v - hi.astype(np.float32)).astype(f16)
        return hi, lo

    in_maps = []
    for c in range(NCORES):
        b, r = divmod(c, 4)
        rows = slice(RB * r, RB * (r + 1))
        xb, gb = x[b], gt[b]
        ni = (xb * xb).sum(-1)
        gi = (gb * gb).sum(-1)

        def packs(coords, nrm, sl):
            m = np.empty((13, RB), f16)
            rh, rl = split(-2.0 * coords[sl].T)
            nh, nl = split(nrm[sl] + 1e-3)  # keeps d^2 > 0 under cancellation
            m[0:3] = rh; m[3:6] = rl; m[6:9] = rh
            m[9] = nh; m[10] = nl; m[11] = 1.0; m[12] = 1.0
            return m

        def packr(coords, nrm):
            m = np.empty((13, A), f16)
            ch, cl = split(coords.T)
            nh, nl = split(nrm)
            m[0:3] = ch; m[3:6] = ch; m[6:9] = cl
            m[9] = 1.0; m[10] = 1.0; m[11] = nh; m[12] = nl
            return m

        mm = np.empty((13, 2 * RB + 2 * A), f16)
        mm[:, 0:RB] = packs(xb, ni, rows)
        mm[:, RB:2 * RB] = packs(gb, gi, rows)
        mm[:, 2 * RB:2 * RB + A] = packr(xb, ni)
        mm[:, 2 * RB + A:2 * RB + 2 * A] = packr(gb, gi)
        in_maps.append({"mm": mm})

    if "nc" not in _CACHE:
        _CACHE["nc"] = _build_bass()
    os.environ.setdefault("BASS_NEVER_TRACE", "1")
    from concourse.bass_utils import run_bass_kernel_spmd
    res = run_bass_kernel_spmd(_CACHE["nc"], in_maps, core_ids=list(range(NCORES)))
    LAST_RESULTS = res
    globals()["LAST_IN_MAPS"] = in_maps

    # Host combine. Device layout: cols [0:4)=s15, [4:8)=s30, [8:12)=s15e,
    # [12:16)=s30e (col index = row-tile t), [16:16+1024) = bond 8-block
    # sums (256 per tile t). Row l = 512*r + 128*t + p.
    loss = 0.0
    for b in range(B):
        s15 = np.empty(A, np.float64); s30 = np.empty(A, np.float64)
        s15e = np.empty(A, np.float64); s30e = np.empty(A, np.float64)
        blk = np.empty((A, T), np.float64)
        for r in range(4):
            o = res.results[4 * b + r]["out"]  # [128, OUTW]
            for seg in range(4):
                base = RB * r + 128 * seg
                s15[base:base + 128] = o[:, seg]
                s30[base:base + 128] = o[:, 4 + seg]
                s15e[base:base + 128] = o[:, 8 + seg]
                s30e[base:base + 128] = o[:, 12 + seg]
                blk[base:base + 128] = o[:, 16 + seg * 256:16 + (seg + 1) * 256]
        bond = (blk * wb_full[b]).sum(-1)
        nuc = is_nuc[b].astype(np.float64)
        c_rows = s15 + nuc * (s30 - s15) - 1.0
        ce_rows = 0.25 * (s15e + nuc * (s30e - s15e)) - E0
        ll = 1.0 - ce_rows.sum() / c_rows.sum()
        a_i = ind[b].T @ gm[b].astype(np.float32)     # atoms per token (masked)
        bond_den = float(a_i @ tbm[b] @ a_i)
        lb = bond.sum() / bond_den
        lm = _mse_host(x[b], gt[b], gm[b], w_atom[b])
        wt = (t[b] ** 2 + SIGMA_DATA ** 2) / (t[b] + SIGMA_DATA) ** 2
        loss += wt * (lm + lb) + ll
    return np.float32(loss / B)
